# revision 39
# baseline (speedup 1.0000x reference)
"""Distributed multi-head attention kernel for one TRN2 chip (8 NeuronCores).

Problem: B=4, N=2048, C=1024, H=16 heads (hd=64), fp32 in/out.
  qkv = x @ W_qkv.T ; per-head scores = q k^T * hd^-0.5 + global_bias
  attn = softmax(scores) ; out = attn @ v ; y = out @ W_proj.T + b_proj

Sharding: head-parallel — core c owns heads {2c, 2c+1} for all batches and
computes qkv projection (its W_qkv rows), attention, and the unnormalized
attention output for its heads over all 8192 tokens.  A single bf16 AllToAll
then redistributes from head-parallel to token-parallel ([8 token slices] x
[128 channels] blocks), after which each core computes the final projection
for its 1024-token slice against the full W_proj.

Everything stays transposed (channels on SBUF partitions) end to end:
  xt [C, B*N], qT/kT [128(2 heads*64), N], v [N, 64] (+ ones column for the
  softmax denominator), out^T [128, B*N], final^T [C, 1024-token slice].
The host prepares transposed/bf16 inputs and untransposes the output;
softmax uses exp(s*scale + b) = exp(s*scale) * eb with eb = exp(bias)
precomputed on the host, so no bias-add pass is needed on-chip.
"""

import numpy as np
import ml_dtypes

import concourse.mybir as mybir
import concourse.tile as tile
from concourse import bacc
from concourse.bass_utils import run_bass_kernel_spmd


def _patch_act_tables():
    """This kernel uses Exp and Ln; by default the table-load pass resolves
    Exp to the `exp_and_others` set and Ln to `natural_log_exp_and_others`,
    thrashing table loads (~1.3us each) between the two.  Hide Exp/the other
    shared fns from every set except `natural_log_exp_and_others` (which has
    both) so a single table load serves the whole kernel."""
    import concourse.hw_specs as hw_specs

    if getattr(bacc, "_act_tables_patched", False):
        return
    orig = hw_specs.get_activation_tables

    def patched(module_arch):
        tables = orig(module_arch)
        keep = tables.get("natural_log_exp_and_others")
        if keep:
            e = mybir.ActivationFunctionType.Exp
            for name, fns in tables.items():
                if name != "natural_log_exp_and_others":
                    fns.discard(e)
        return tables

    bacc.get_activation_tables = patched
    bacc._act_tables_patched = True


_patch_act_tables()

F32 = mybir.dt.float32
BF16 = mybir.dt.bfloat16
I16 = mybir.dt.int16
BF16_NP = ml_dtypes.bfloat16

N_CORES = 8
B, N, C = 4, 2048, 1024
H = 16
HD = C // H          # 64
SCALE = HD ** -0.5
# Schraudolph bf16 exp: bits(exp(scale*s + bias)) ~ A*s + LB with
# A = 128*log2(e)*scale and LB = 16256 - C_ADJ + 128*log2(e)*bias (int16);
# used on even k-tiles, exact exp on odd ones (error ~1.3e-2 < 2e-2 gate)
A_EXP = float(128 * np.log2(np.e) * SCALE)
C_ADJ = 5.5
TOK = B * N          # 8192
TSLICE = TOK // N_CORES  # 1024 tokens per core for the final projection
NCT = C // 128       # 8 c-tiles
NKT = N // 128       # 16 k-tiles per batch
NQC = N // 512       # 4 q-chunks per batch
GK = 2               # k-tiles per exp group
TB = TSLICE // B     # 256 tokens per (core, batch) in the final output

_GRAPH = None


def _build():
    nc = bacc.Bacc("TRN2", target_bir_lowering=False, debug=False,
                   num_devices=N_CORES)

    xt = nc.declare_dram_parameter("xt", [C, TOK], BF16, isOutput=False)
    wq = nc.declare_dram_parameter("wq", [C, 128], BF16, isOutput=False)
    wk = nc.declare_dram_parameter("wk", [C, 128], BF16, isOutput=False)
    wv = nc.declare_dram_parameter("wv", [C, 130], BF16, isOutput=False)
    wp = nc.declare_dram_parameter("wp", [C, C], BF16, isOutput=False)
    bp = nc.declare_dram_parameter("bp", [C, 1], F32, isOutput=False)
    lb = nc.declare_dram_parameter("lb", [128, NKT // 2, N], I16,
                                   isOutput=False)
    eb = nc.declare_dram_parameter("eb", [128, NKT // 2, N], BF16,
                                   isOutput=False)
    out = nc.declare_dram_parameter("out", [C, TSLICE], F32, isOutput=True)

    xt_r = xt.rearrange("(ct p) t -> p ct t", p=128)
    wq_r = wq.rearrange("(ct p) f -> p ct f", p=128)
    wk_r = wk.rearrange("(ct p) f -> p ct f", p=128)
    wv_r = wv.rearrange("(ct p) f -> p ct f", p=128)
    wp_r = wp.rearrange("(ct p) o -> p ct o", p=128)
    bp_r = bp.rearrange("(ot p) one -> p ot one", p=128)

    with tile.TileContext(nc) as tc:
        with (
            tc.tile_pool(name="const", bufs=1) as cpool,
            tc.tile_pool(name="xt", bufs=1) as xpool,
            tc.tile_pool(name="qk", bufs=2) as qkpool,
            tc.tile_pool(name="vv", bufs=2) as vpool,
            tc.tile_pool(name="pp", bufs=5) as ppool,
            tc.tile_pool(name="pe", bufs=3) as pepool,
            tc.tile_pool(name="outu", bufs=1) as opool,
            tc.tile_pool(name="den", bufs=1) as dpool,
            tc.tile_pool(name="outn", bufs=1) as onpool,
            tc.tile_pool(name="fin", bufs=2) as fpool,
            tc.tile_pool(name="gat", bufs=1) as gpool,
            tc.tile_pool(name="dram", bufs=1, space="DRAM") as drpool,
            tc.tile_pool(name="ps_s", bufs=2, space="PSUM") as ps_s,
            tc.tile_pool(name="ps_o", bufs=2, space="PSUM") as ps_o,
            tc.tile_pool(name="ps_m", bufs=2, space="PSUM") as ps_m,
        ):
            # ---- resident constants -------------------------------------
            # qkv weights first: batch 0's qkv is the critical path at start;
            # eb/wp are not needed until attention / the first projection
            wq_t = cpool.tile([128, NCT, 128], BF16, tag="wq")
            wk_t = cpool.tile([128, NCT, 128], BF16, tag="wk")
            wv_t = cpool.tile([128, NCT, 130], BF16, tag="wv")
            nc.sync.dma_start(wq_t[:], wq_r)
            nc.sync.dma_start(wk_t[:], wk_r)
            nc.sync.dma_start(wv_t[:], wv_r)
            bp_t = cpool.tile([128, NCT, 1], F32, tag="bp")
            nc.sync.dma_start(bp_t[:], bp_r)
            ones_t = cpool.tile([1, 64], F32, tag="ones")
            nc.gpsimd.memset(ones_t[:], 1.0)
            # indicator rows: ind_h is 1 on head h's 64-partition range, so
            # ind_h^T @ rc_h broadcasts the 1/den row to those partitions;
            # two accumulating K=1 matmuls fill all 128 partitions of one
            # psum bank (bf16 moving, ~4x faster than the old f32 ones-mm)
            ind0 = cpool.tile([1, 128], BF16, tag="ind0")
            ind1 = cpool.tile([1, 128], BF16, tag="ind1")
            nc.gpsimd.memset(ind0[0:1, 0:64], 1.0)
            nc.gpsimd.memset(ind0[0:1, 64:128], 0.0)
            nc.gpsimd.memset(ind1[0:1, 0:64], 0.0)
            nc.gpsimd.memset(ind1[0:1, 64:128], 1.0)
            # touch Exp now so the ~2.7us activation table load happens
            # under the initial DMAs, not at the first real softmax
            wexp_t = cpool.tile([1, 64], F32, tag="wexp")
            nc.scalar.activation(
                wexp_t[:], ones_t[:], mybir.ActivationFunctionType.Exp
            )
            # dummy matmuls while the x DMAs land: ~5us of sustained PE
            # activity flips the HAM clock gate to 8/8 so the first real
            # qkv chains run at full rate instead of 1.2GHz
            wrm_t = cpool.tile([1, 512], BF16, tag="wrm")
            nc.vector.memset(wrm_t[:], 0.0)
            for _w in range(12):
                pwarm = ps_m.tile([128, 512], F32, tag="ps_m")
                nc.tensor.matmul(pwarm[:], ind0[:], wrm_t[:],
                                 start=True, stop=True)
            lb_ts = []
            eb_ts = []
            for j in range(NKT // 2):
                lbj = cpool.tile([128, N], I16, tag=f"lb{j}")
                lb_ts.append(lbj)
            for j in range(NKT // 2):
                ebj = cpool.tile([128, N], BF16, tag=f"eb{j}")
                eb_ts.append(ebj)
            wp_t = cpool.tile([128, NCT, C], BF16, tag="wp")

            # warmup collective: absorb the first-call ENCD/NCCL staging
            # latency (~40us) while the initial DMAs and QKV run.  Tiny
            # payload: the staging cost is fixed, and the collectives sit at
            # the head of the gpsimd FIFO - nothing critical may queue
            # behind them until they finish
            wu_i = drpool.tile([N_CORES, 128, 8], BF16, tag="wu_i")
            wu_o = drpool.tile([N_CORES, 128, 8], BF16, tag="wu_o")
            wz = cpool.tile([128, 8], BF16, tag="wz")
            nc.gpsimd.memset(wz[:], 0.0)
            nc.sync.dma_start(wu_i[0, :, :], wz[:])
            for _wu in range(2):
                nc.gpsimd.collective_compute(
                    "AllToAll",
                    mybir.AluOpType.bypass,
                    replica_groups=[list(range(N_CORES))],
                    ins=[wu_i.opt()],
                    outs=[wu_o.opt()],
                )

            def emit_proj_part(a2a_o_, out_c0, tb, ots, gat_tag="gat"):
                """Gather an AllToAll result and project output tiles `ots`
                of it into out[:, out_c0:out_c0+tb]."""
                if ots[0] == 0:
                    gat = gpool.tile([128, NCT, tb], BF16, tag=gat_tag)
                    for ct in range(NCT):
                        nc.sync.dma_start(gat[:, ct, :], a2a_o_[ct, :, :])
                    emit_proj_part.gat[gat_tag] = gat
                else:
                    gat = emit_proj_part.gat[gat_tag]
                for ot in ots:
                    pf = ps_m.tile([128, tb], F32, tag="ps_m")
                    for ct in range(NCT):
                        nc.tensor.matmul(
                            pf[:],
                            wp_t[:, ct, ot * 128:(ot + 1) * 128],
                            gat[:, ct, :],
                            start=(ct == 0), stop=(ct == NCT - 1),
                        )
                    fin = fpool.tile([128, tb], F32, tag="fin")
                    nc.vector.tensor_scalar_add(fin[:], pf[:], bp_t[:, ot, :])
                    nc.sync.dma_start(
                        out[ot * 128:(ot + 1) * 128, out_c0:out_c0 + tb],
                        fin[:],
                    )

            emit_proj_part.gat = {}

            def emit_proj(pb_, a2a_o_):
                emit_proj_part(a2a_o_, pb_ * TB, TB, list(range(NCT)))

            pending_proj = None

            xt_tiles = {}

            def load_xt(bb, chunked=False):
                xt_t = xpool.tile([128, NCT, N], BF16, tag="xt")
                if chunked:
                    # column-major chunks so the first qkv chain (which needs
                    # all 8 c-tiles of columns 0:512) waits on 1MB, not 4MB
                    for tcn in range(NQC):
                        for ct in range(NCT):
                            nc.sync.dma_start(
                                xt_t[:, ct, tcn * 512:(tcn + 1) * 512],
                                xt_r[:, ct,
                                     bb * N + tcn * 512:
                                     bb * N + (tcn + 1) * 512],
                            )
                else:
                    for ct in range(NCT):
                        nc.sync.dma_start(
                            xt_t[:, ct, :], xt_r[:, ct, bb * N:(bb + 1) * N]
                        )
                xt_tiles[bb] = xt_t

            load_xt(0, chunked=True)

            qkv_tiles = {}

            def alloc_qkv(bb):
                qT = qkpool.tile([128, N], BF16, tag="qT")
                kT = qkpool.tile([128, N], BF16, tag="kT")
                v_t = vpool.tile([128, NKT, 130], BF16, tag="vv")
                # softmax-denominator ones columns for every k-tile in two
                # strided memsets (v_chain's psum drains skip these columns)
                nc.vector.memset(v_t[:, :, 64:65], 1.0)
                nc.vector.memset(v_t[:, :, 129:130], 1.0)
                qkv_tiles[bb] = (qT, kT, v_t)

            def qkv_chunks(bb):
                """Yield thunks, each emitting one 8-matmul qkv chain for
                batch bb.  Emitted interleaved into the previous batch's
                attention so the PE always has dense independent work."""
                qT, kT, v_t = qkv_tiles[bb]
                xt_t = xt_tiles[bb]

                def qk_chain(dst, w_t, tcn):
                    pqk = ps_m.tile([128, 512], F32, tag="ps_m")
                    for ct in range(NCT):
                        nc.tensor.matmul(
                            pqk[:],
                            w_t[:, ct, :],
                            xt_t[:, ct, tcn * 512:(tcn + 1) * 512],
                            start=(ct == 0), stop=(ct == NCT - 1),
                        )
                    # psum drains ride the scalar engine (ACT is close to
                    # PSUM and has slack; DVE carries the softmax ssts)
                    nc.scalar.copy(
                        dst[:, tcn * 512:(tcn + 1) * 512], pqk[:]
                    )

                def v_chain(kt):
                    # v (+ones cols): head slices [0:65]=[v_h0|ones] and
                    # [65:130]=[v_h1|ones] put both denominators at psum
                    # row 64
                    pv = ps_m.tile([128, 512], F32, tag="ps_m")
                    for ct in range(NCT):
                        nc.tensor.matmul(
                            pv[:, 0:130],
                            xt_t[:, ct, kt * 128:(kt + 1) * 128],
                            wv_t[:, ct, :],
                            start=(ct == 0), stop=(ct == NCT - 1),
                        )
                    nc.scalar.copy(v_t[:, kt, 0:64], pv[:, 0:64])
                    nc.scalar.copy(v_t[:, kt, 65:129], pv[:, 65:129])

                for dst, w_t in ((qT, wq_t), (kT, wk_t)):
                    for tcn in range(NQC):
                        yield lambda d=dst, w=w_t, t=tcn: qk_chain(d, w, t)
                for kt in range(NKT):
                    yield lambda k=kt: v_chain(k)

            # batch 0's qkv runs up front (nothing to overlap with)
            alloc_qkv(0)
            for chunk in qkv_chunks(0):
                chunk()
            if B > 1:
                load_xt(1, chunked=True)
                alloc_qkv(1)
            # bias tables land on the hw-dma queues behind the xt loads:
            # lb/eb tile j is first needed at attention(0) k-tile 2j (~55us
            # in), well after these ~8MB drain; wp only at the first
            # projection (end of batch 1)
            for j in range(NKT // 2):
                nc.sync.dma_start(lb_ts[j][:], lb[:, j, :])
                nc.sync.dma_start(eb_ts[j][:], eb[:, j, :])
            for ct in range(NCT):
                nc.sync.dma_start(wp_t[:, ct, :], wp_r[:, ct, :])

            # ---- per-batch attention ------------------------------------
            for b in range(B):
                qT, kT, v_t = qkv_tiles.pop(b)
                xt_tiles.pop(b)
                filler = iter(qkv_chunks(b + 1)) if b + 1 < B else iter(())

                def emit_filler(n=1):
                    for _ in range(n):
                        ch = next(filler, None)
                        if ch is not None:
                            ch()

                ln0 = dpool.tile([1, N], F32, tag="ln0")
                ln1 = dpool.tile([1, N], F32, tag="ln1")
                rc0 = dpool.tile([1, N], BF16, tag="rc0")
                rc1 = dpool.tile([1, N], BF16, tag="rc1")
                ou = opool.tile([128, N], BF16, tag="outu")
                on = onpool.tile([128, N], BF16, tag="outn")
                last = b == B - 1
                if last:
                    # the last batch ships in three AllToAlls (qc0-1 as
                    # 128-token chunks, then qc2 and qc3 as 64-token chunks)
                    # so only one small collective remains after attention
                    a2a_i3 = []
                    a2a_o3 = []
                    for hf, w in ((0, 128), (1, 64), (2, 64)):
                        a2a_i3h = drpool.tile([N_CORES, 128, w], BF16,
                                              tag=f"a2ai{b}_{hf}")
                        a2a_o3h = drpool.tile([N_CORES, 128, w], BF16,
                                              tag=f"a2ao{b}_{hf}")
                        a2a_i3.append(a2a_i3h)
                        a2a_o3.append(a2a_o3h)

                    def issue_a2a3(hf):
                        nc.gpsimd.collective_compute(
                            "AllToAll",
                            mybir.AluOpType.bypass,
                            replica_groups=[list(range(N_CORES))],
                            ins=[a2a_i3[hf].opt()],
                            outs=[a2a_o3[hf].opt()],
                        )
                else:
                    a2a_i = drpool.tile([N_CORES, 128, TB], BF16,
                                        tag=f"a2ai{b}")
                    a2a_o = drpool.tile([N_CORES, 128, TB], BF16,
                                        tag=f"a2ao{b}")
                # Both heads processed together per k-tile: the two K=64
                # score matmuls occupy disjoint PE row groups (partitions
                # 0-63 / 64-127) and run concurrently.  attn@v matmuls are
                # skewed one k-tile behind the scores so the PE never waits
                # on the exp->mul chain of the current k-tile.
                for qc in range(NQC):
                    q0 = qc * 512
                    po0 = ps_o.tile([65, 512], F32, tag="ps_o")
                    po1 = ps_o.tile([65, 512], F32, tag="ps_o")
                    po = [po0, po1]
                    pend = []
                    for kt in range(NKT):
                        ps = ps_s.tile([128, 1024], F32, tag="ps_s")
                        for h in range(2):
                            nc.tensor.matmul(
                                ps[:, h * 512:(h + 1) * 512],
                                kT[h * 64:h * 64 + 64,
                                   kt * 128:(kt + 1) * 128],
                                qT[h * 64:h * 64 + 64, q0:q0 + 512],
                                start=True, stop=True,
                            )
                        if len(pend) >= 3:
                            pkt, ppw = pend.pop(0)
                            for h in range(2):
                                nc.tensor.matmul(
                                    po[h][:],
                                    v_t[:, pkt, h * 65:h * 65 + 65],
                                    ppw[:, h * 512:(h + 1) * 512],
                                    start=(pkt == 0), stop=False,
                                )
                        pw = ppool.tile([128, 1024], BF16, tag="pp")
                        if kt % 2 == 0:
                            # Schraudolph: pw bits = int16(ps*A + LB), read
                            # back as bf16 = approx exp(scale*s + bias) with
                            # exp(bias) folded into the int table; one wide
                            # DVE op, LB broadcast over the two heads
                            lbs = (lb_ts[kt // 2][:, q0:q0 + 512]
                                   .unsqueeze(1).broadcast_to([128, 2, 512]))
                            nc.vector.scalar_tensor_tensor(
                                pw[:].bitcast(I16), ps[:], A_EXP, lbs,
                                mybir.AluOpType.mult, mybir.AluOpType.add,
                            )
                        else:
                            # exact path: table exp then *exp(bias); both
                            # multiplies stay on DVE - gpsimd elementwise is
                            # ~3x slower and steals the DVE's SBUF port, and
                            # anything queued on gpsimd behind a collective
                            # head-of-line blocks attention
                            pexp = pepool.tile([128, 1024], BF16, tag="pe")
                            nc.scalar.activation(
                                pexp[:], ps[:],
                                mybir.ActivationFunctionType.Exp,
                                scale=SCALE,
                            )
                            ebs = eb_ts[kt // 2][:, q0:q0 + 512]
                            for h in range(2):
                                nc.vector.tensor_mul(
                                    pw[:, h * 512:(h + 1) * 512],
                                    pexp[:, h * 512:(h + 1) * 512],
                                    ebs,
                                )
                        pend.append((kt, pw))
                        # interleave next batch's qkv chains as PE filler
                        if (qc == 0 and kt >= 9 and kt % 2 == 1) or \
                           (qc > 0 and kt % 3 == 2):
                            emit_filler(1)
                    for pkt, ppw in pend:
                        for h in range(2):
                            nc.tensor.matmul(
                                po[h][:],
                                v_t[:, pkt, h * 65:h * 65 + 65],
                                ppw[:, h * 512:(h + 1) * 512],
                                start=False, stop=(pkt == NKT - 1),
                            )
                    for h, lnd, rcd in ((0, ln0, rc0), (1, ln1, rc1)):
                        nc.vector.tensor_copy(
                            ou[h * 64:(h + 1) * 64, q0:q0 + 512],
                            po[h][0:64, :],
                        )
                        nc.scalar.activation(
                            lnd[0:1, q0:q0 + 512], po[h][64:65, :],
                            mybir.ActivationFunctionType.Ln,
                        )
                        # 1/den = exp(-ln den), one row per head
                        nc.scalar.activation(
                            rcd[0:1, q0:q0 + 512], lnd[0:1, q0:q0 + 512],
                            mybir.ActivationFunctionType.Exp, scale=-1.0,
                        )
                    # broadcast each head's 1/den row to its 64-partition
                    # range via two accumulating K=1 indicator matmuls, then
                    # normalize reading the broadcast straight from PSUM
                    pb = ps_o.tile([128, 512], F32, tag="ps_o")
                    nc.tensor.matmul(
                        pb[:], ind0[:], rc0[0:1, q0:q0 + 512],
                        start=True, stop=False,
                    )
                    nc.tensor.matmul(
                        pb[:], ind1[:], rc1[0:1, q0:q0 + 512],
                        start=False, stop=True,
                    )
                    nc.vector.tensor_mul(
                        on[:, q0:q0 + 512], ou[:, q0:q0 + 512], pb[:]
                    )
                    if last:
                        if qc < 2:
                            for j in range(4):
                                nc.sync.dma_start(
                                    a2a_i3[0][qc * 4 + j, :, :],
                                    on[:, q0 + j * 128:q0 + (j + 1) * 128],
                                )
                            if qc == 1:
                                issue_a2a3(0)
                        else:
                            for j in range(N_CORES):
                                nc.sync.dma_start(
                                    a2a_i3[qc - 1][j, :, :],
                                    on[:, q0 + j * 64:q0 + (j + 1) * 64],
                                )
                            issue_a2a3(qc - 1)
                        # the previous batch's projection fills the
                        # filler-less last batch late enough that its
                        # AllToAll is long finished even in the scheduler's
                        # optimistic timeline; this batch's qc0-1 projection
                        # slots under qc3
                        if pending_proj is not None and qc in (2, 3):
                            emit_proj_part(
                                pending_proj[1], pending_proj[0] * TB, TB,
                                [0, 1, 2, 3] if qc == 2 else [4, 5, 6, 7],
                            )
                            if qc == 3:
                                pending_proj = None
                        if qc == 3:
                            emit_proj_part(a2a_o3[0], b * TB, 128,
                                           list(range(NCT)), gat_tag="gat3")
                    else:
                        for j in (2 * qc, 2 * qc + 1):
                            nc.sync.dma_start(
                                a2a_i[j, :, :], on[:, j * TB:(j + 1) * TB]
                            )
                        if qc == 3 and pending_proj is not None:
                            # previous batch's projection, emitted as late as
                            # possible: its gather depends on that batch's
                            # AllToAll, and scheduling it early lets the PE's
                            # static instruction order head-of-line block on
                            # the collective at the batch boundary
                            emit_proj(*pending_proj)
                            pending_proj = None
                    emit_filler(2)

                emit_filler(NKT + 2 * NQC)  # flush any remaining chunks
                if not last:
                    nc.gpsimd.collective_compute(
                        "AllToAll",
                        mybir.AluOpType.bypass,
                        replica_groups=[list(range(N_CORES))],
                        ins=[a2a_i.opt()],
                        outs=[a2a_o.opt()],
                    )
                    pending_proj = (b, a2a_o)
                if b + 2 < B:
                    load_xt(b + 2)
                    alloc_qkv(b + 2)

            emit_proj_part(a2a_o3[1], (B - 1) * TB + 128, 64,
                           list(range(NCT)), gat_tag="gat3b")
            emit_proj_part(a2a_o3[2], (B - 1) * TB + 192, 64,
                           list(range(NCT)), gat_tag="gat3c")
    nc.compile()
    return nc


def _graph():
    global _GRAPH
    if _GRAPH is None:
        _GRAPH = _build()
    return _GRAPH


def _prep_inputs(x, W_qkv, W_proj, b_proj, global_bias):
    x = np.asarray(x, dtype=np.float32)
    W_qkv = np.asarray(W_qkv, dtype=np.float32)
    W_proj = np.asarray(W_proj, dtype=np.float32)
    b_proj = np.asarray(b_proj, dtype=np.float32)
    global_bias = np.asarray(global_bias, dtype=np.float32)

    xt = np.ascontiguousarray(x.reshape(TOK, C).T).astype(BF16_NP)
    wp = np.ascontiguousarray(W_proj.T).astype(BF16_NP)
    bpv = np.ascontiguousarray(b_proj[:, None])
    ebt = np.exp(global_bias).T  # [k, q]
    eb_kt = ebt.reshape(NKT, 128, N)
    ebp = np.ascontiguousarray(
        eb_kt[1::2].transpose(1, 0, 2)
    ).astype(BF16_NP)
    # Schraudolph int table for even k-tiles: bf16 bits of exp(bias) shifted
    # into the exponent field, to be added to A_EXP*scores
    lbt = np.round(
        16256.0 - C_ADJ + (128 * np.log2(np.e)) * global_bias
    ).astype(np.int16).T  # [k, q]
    lbp = np.ascontiguousarray(
        lbt.reshape(NKT, 128, N)[0::2].transpose(1, 0, 2)
    )

    in_maps = []
    for c in range(N_CORES):
        r0 = c * 128
        wq_c = np.ascontiguousarray(W_qkv[r0:r0 + 128, :].T).astype(BF16_NP)
        wk_c = np.ascontiguousarray(W_qkv[C + r0:C + r0 + 128, :].T).astype(BF16_NP)
        vt = W_qkv[2 * C + r0:2 * C + r0 + 128, :].T  # [C, 128]
        wv_c = np.zeros((C, 130), dtype=np.float32)
        wv_c[:, 0:64] = vt[:, 0:64]
        wv_c[:, 65:129] = vt[:, 64:128]
        in_maps.append({
            "xt": xt,
            "wq": wq_c,
            "wk": wk_c,
            "wv": wv_c.astype(BF16_NP),
            "wp": wp,
            "bp": bpv,
            "lb": lbp,
            "eb": ebp,
        })
    return in_maps


def _assemble(results):
    full = np.empty((TOK, C), dtype=np.float32)
    for c in range(N_CORES):
        o = results[c]["out"].T  # [4*TB tokens, C]
        for b in range(B - 1):
            full[b * N + c * TB:b * N + (c + 1) * TB, :] = (
                o[b * TB:(b + 1) * TB, :]
            )
        # last batch shipped in three AllToAlls: qc0-1 as 128-token
        # chunks, qc2 and qc3 as 64-token chunks
        b = B - 1
        full[b * N + c * 128:b * N + (c + 1) * 128, :] = (
            o[b * TB:b * TB + 128, :]
        )
        full[b * N + 1024 + c * 64:b * N + 1024 + (c + 1) * 64, :] = (
            o[b * TB + 128:b * TB + 192, :]
        )
        full[b * N + 1536 + c * 64:b * N + 1536 + (c + 1) * 64, :] = (
            o[b * TB + 192:b * TB + 256, :]
        )
    return full.reshape(B, N, C)


def kernel(x, W_qkv, W_proj, b_proj, global_bias):
    nc = _graph()
    in_maps = _prep_inputs(x, W_qkv, W_proj, b_proj, global_bias)
    res = run_bass_kernel_spmd(nc, in_maps, core_ids=list(range(N_CORES)))
    return _assemble(res.results)


def run_profiled(x, W_qkv, W_proj, b_proj, global_bias, **trace_kwargs):
    """Like kernel() but with NTFF profiling; returns (output, results)."""
    nc = _graph()
    in_maps = _prep_inputs(x, W_qkv, W_proj, b_proj, global_bias)
    res = run_bass_kernel_spmd(
        nc, in_maps, core_ids=list(range(N_CORES)), trace=True, **trace_kwargs
    )
    return _assemble(res.results), res



# revision 43
# speedup vs baseline: 1.0446x; 1.0446x over previous
"""Distributed multi-head attention kernel for one TRN2 chip (8 NeuronCores).

Problem: B=4, N=2048, C=1024, H=16 heads (hd=64), fp32 in/out.
  qkv = x @ W_qkv.T ; per-head scores = q k^T * hd^-0.5 + global_bias
  attn = softmax(scores) ; out = attn @ v ; y = out @ W_proj.T + b_proj

Sharding: head-parallel — core c owns heads {2c, 2c+1} for all batches and
computes qkv projection (its W_qkv rows), attention, and the unnormalized
attention output for its heads over all 8192 tokens.  A single bf16 AllToAll
then redistributes from head-parallel to token-parallel ([8 token slices] x
[128 channels] blocks), after which each core computes the final projection
for its 1024-token slice against the full W_proj.

Everything stays transposed (channels on SBUF partitions) end to end:
  xt [C, B*N], qT/kT [128(2 heads*64), N], v [N, 64] (+ ones column for the
  softmax denominator), out^T [128, B*N], final^T [C, 1024-token slice].
The host prepares transposed/bf16 inputs and untransposes the output;
softmax uses exp(s*scale + b) = exp(s*scale) * eb with eb = exp(bias)
precomputed on the host, so no bias-add pass is needed on-chip.
"""

import numpy as np
import ml_dtypes

import concourse.mybir as mybir
import concourse.tile as tile
from concourse import bacc
from concourse.bass_utils import run_bass_kernel_spmd


def _patch_act_tables():
    """This kernel uses Exp and Ln; by default the table-load pass resolves
    Exp to the `exp_and_others` set and Ln to `natural_log_exp_and_others`,
    thrashing table loads (~1.3us each) between the two.  Hide Exp/the other
    shared fns from every set except `natural_log_exp_and_others` (which has
    both) so a single table load serves the whole kernel."""
    import concourse.hw_specs as hw_specs

    if getattr(bacc, "_act_tables_patched", False):
        return
    orig = hw_specs.get_activation_tables

    def patched(module_arch):
        tables = orig(module_arch)
        keep = tables.get("natural_log_exp_and_others")
        if keep:
            e = mybir.ActivationFunctionType.Exp
            for name, fns in tables.items():
                if name != "natural_log_exp_and_others":
                    fns.discard(e)
        return tables

    bacc.get_activation_tables = patched
    bacc._act_tables_patched = True


_patch_act_tables()

F32 = mybir.dt.float32
BF16 = mybir.dt.bfloat16
I16 = mybir.dt.int16
BF16_NP = ml_dtypes.bfloat16

N_CORES = 8
B, N, C = 4, 2048, 1024
H = 16
HD = C // H          # 64
SCALE = HD ** -0.5
# Schraudolph bf16 exp: bits(exp(scale*s + bias)) ~ A*s + LB with
# A = 128*log2(e)*scale and LB = 16256 - C_ADJ + 128*log2(e)*bias (int16);
# used on even k-tiles, exact exp on odd ones (error ~1.3e-2 < 2e-2 gate)
A_EXP = float(128 * np.log2(np.e) * SCALE)
C_ADJ = 5.5
TOK = B * N          # 8192
TSLICE = TOK // N_CORES  # 1024 tokens per core for the final projection
NCT = C // 128       # 8 c-tiles
NKT = N // 128       # 16 k-tiles per batch
NQC = N // 512       # 4 q-chunks per batch
GK = 2               # k-tiles per exp group
TB = TSLICE // B     # 256 tokens per (core, batch) in the final output

_GRAPH = None


def _build():
    nc = bacc.Bacc("TRN2", target_bir_lowering=False, debug=False,
                   num_devices=N_CORES)

    xt = nc.declare_dram_parameter("xt", [C, TOK], BF16, isOutput=False)
    wq = nc.declare_dram_parameter("wq", [C, 128], BF16, isOutput=False)
    wk = nc.declare_dram_parameter("wk", [C, 128], BF16, isOutput=False)
    wv = nc.declare_dram_parameter("wv", [C, 130], BF16, isOutput=False)
    wp = nc.declare_dram_parameter("wp", [C, C], BF16, isOutput=False)
    bp = nc.declare_dram_parameter("bp", [C, 1], F32, isOutput=False)
    lb = nc.declare_dram_parameter("lb", [128, NKT // 2, N], I16,
                                   isOutput=False)
    eb = nc.declare_dram_parameter("eb", [128, NKT // 2, N], BF16,
                                   isOutput=False)
    out = nc.declare_dram_parameter("out", [C, TSLICE], F32, isOutput=True)

    xt_r = xt.rearrange("(ct p) t -> p ct t", p=128)
    wq_r = wq.rearrange("(ct p) f -> p ct f", p=128)
    wk_r = wk.rearrange("(ct p) f -> p ct f", p=128)
    wv_r = wv.rearrange("(ct p) f -> p ct f", p=128)
    wp_r = wp.rearrange("(ct p) o -> p ct o", p=128)
    bp_r = bp.rearrange("(ot p) one -> p ot one", p=128)

    with tile.TileContext(nc) as tc:
        with (
            tc.tile_pool(name="const", bufs=1) as cpool,
            tc.tile_pool(name="xt", bufs=1) as xpool,
            tc.tile_pool(name="qk", bufs=2) as qkpool,
            tc.tile_pool(name="vv", bufs=2) as vpool,
            tc.tile_pool(name="pp", bufs=5) as ppool,
            tc.tile_pool(name="pe", bufs=3) as pepool,
            tc.tile_pool(name="outu", bufs=1) as opool,
            tc.tile_pool(name="den", bufs=1) as dpool,
            tc.tile_pool(name="outn", bufs=1) as onpool,
            tc.tile_pool(name="fin", bufs=2) as fpool,
            tc.tile_pool(name="gat", bufs=1) as gpool,
            tc.tile_pool(name="dram", bufs=1, space="DRAM") as drpool,
            tc.tile_pool(name="ps_s", bufs=2, space="PSUM") as ps_s,
            tc.tile_pool(name="ps_o", bufs=2, space="PSUM") as ps_o,
            tc.tile_pool(name="ps_m", bufs=2, space="PSUM") as ps_m,
        ):
            # ---- resident constants -------------------------------------
            # qkv weights first: batch 0's qkv is the critical path at start;
            # eb/wp are not needed until attention / the first projection
            wq_t = cpool.tile([128, NCT, 128], BF16, tag="wq")
            wk_t = cpool.tile([128, NCT, 128], BF16, tag="wk")
            wv_t = cpool.tile([128, NCT, 130], BF16, tag="wv")
            nc.sync.dma_start(wq_t[:], wq_r)
            nc.sync.dma_start(wk_t[:], wk_r)
            nc.sync.dma_start(wv_t[:], wv_r)
            bp_t = cpool.tile([128, NCT, 1], F32, tag="bp")
            nc.sync.dma_start(bp_t[:], bp_r)
            ones_t = cpool.tile([1, 64], F32, tag="ones")
            nc.gpsimd.memset(ones_t[:], 1.0)
            # indicator rows: ind_h is 1 on head h's 64-partition range, so
            # ind_h^T @ rc_h broadcasts the 1/den row to those partitions;
            # two accumulating K=1 matmuls fill all 128 partitions of one
            # psum bank (bf16 moving, ~4x faster than the old f32 ones-mm)
            ind0 = cpool.tile([1, 128], BF16, tag="ind0")
            ind1 = cpool.tile([1, 128], BF16, tag="ind1")
            nc.gpsimd.memset(ind0[0:1, 0:64], 1.0)
            nc.gpsimd.memset(ind0[0:1, 64:128], 0.0)
            nc.gpsimd.memset(ind1[0:1, 0:64], 0.0)
            nc.gpsimd.memset(ind1[0:1, 64:128], 1.0)
            # touch Exp now so the ~2.7us activation table load happens
            # under the initial DMAs, not at the first real softmax
            wexp_t = cpool.tile([1, 64], F32, tag="wexp")
            nc.scalar.activation(
                wexp_t[:], ones_t[:], mybir.ActivationFunctionType.Exp
            )
            # dummy matmuls while the x DMAs land: ~5us of sustained PE
            # activity flips the HAM clock gate to 8/8 so the first real
            # qkv chains run at full rate instead of 1.2GHz
            wrm_t = cpool.tile([1, 512], BF16, tag="wrm")
            nc.vector.memset(wrm_t[:], 0.0)
            for _w in range(12):
                pwarm = ps_m.tile([128, 512], F32, tag="ps_m")
                nc.tensor.matmul(pwarm[:], ind0[:], wrm_t[:],
                                 start=True, stop=True)
            lb_ts = []
            eb_ts = []
            for j in range(NKT // 2):
                lbj = cpool.tile([128, N], I16, tag=f"lb{j}")
                lb_ts.append(lbj)
            for j in range(NKT // 2):
                ebj = cpool.tile([128, N], BF16, tag=f"eb{j}")
                eb_ts.append(ebj)
            wp_t = cpool.tile([128, NCT, C], BF16, tag="wp")

            # warmup collective: absorb the first-call ENCD/NCCL staging
            # latency (~40us) while the initial DMAs and QKV run.  Tiny
            # payload: the staging cost is fixed, and the collectives sit at
            # the head of the gpsimd FIFO - nothing critical may queue
            # behind them until they finish
            wu_i = drpool.tile([N_CORES, 128, 8], BF16, tag="wu_i")
            wu_o = drpool.tile([N_CORES, 128, 8], BF16, tag="wu_o")
            wz = cpool.tile([128, 8], BF16, tag="wz")
            nc.gpsimd.memset(wz[:], 0.0)
            nc.sync.dma_start(wu_i[0, :, :], wz[:])
            for _wu in range(2):
                nc.gpsimd.collective_compute(
                    "AllToAll",
                    mybir.AluOpType.bypass,
                    replica_groups=[list(range(N_CORES))],
                    ins=[wu_i.opt()],
                    outs=[wu_o.opt()],
                )

            def emit_proj_part(a2a_o_, out_c0, tb, ots, gat_tag="gat"):
                """Gather an AllToAll result and project output tiles `ots`
                of it into out[:, out_c0:out_c0+tb]."""
                if ots[0] == 0:
                    gat = gpool.tile([128, NCT, tb], BF16, tag=gat_tag)
                    for ct in range(NCT):
                        nc.sync.dma_start(gat[:, ct, :], a2a_o_[ct, :, :])
                    emit_proj_part.gat[gat_tag] = gat
                else:
                    gat = emit_proj_part.gat[gat_tag]
                for ot in ots:
                    pf = ps_m.tile([128, tb], F32, tag="ps_m")
                    for ct in range(NCT):
                        nc.tensor.matmul(
                            pf[:],
                            wp_t[:, ct, ot * 128:(ot + 1) * 128],
                            gat[:, ct, :],
                            start=(ct == 0), stop=(ct == NCT - 1),
                        )
                    fin = fpool.tile([128, tb], F32, tag="fin")
                    nc.vector.tensor_scalar_add(fin[:], pf[:], bp_t[:, ot, :])
                    nc.sync.dma_start(
                        out[ot * 128:(ot + 1) * 128, out_c0:out_c0 + tb],
                        fin[:],
                    )

            emit_proj_part.gat = {}

            def emit_proj(pb_, a2a_o_):
                emit_proj_part(a2a_o_, pb_ * TB, TB, list(range(NCT)))

            pending_proj = None

            xt_tiles = {}

            def load_xt(bb, chunked=False):
                xt_t = xpool.tile([128, NCT, N], BF16, tag="xt")
                if chunked:
                    # column-major chunks so the first qkv chain (which needs
                    # all 8 c-tiles of columns 0:512) waits on 1MB, not 4MB
                    for tcn in range(NQC):
                        for ct in range(NCT):
                            nc.sync.dma_start(
                                xt_t[:, ct, tcn * 512:(tcn + 1) * 512],
                                xt_r[:, ct,
                                     bb * N + tcn * 512:
                                     bb * N + (tcn + 1) * 512],
                            )
                else:
                    for ct in range(NCT):
                        nc.sync.dma_start(
                            xt_t[:, ct, :], xt_r[:, ct, bb * N:(bb + 1) * N]
                        )
                xt_tiles[bb] = xt_t

            load_xt(0, chunked=True)

            qkv_tiles = {}

            def alloc_qkv(bb):
                qT = qkpool.tile([128, N], BF16, tag="qT")
                kT = qkpool.tile([128, N], BF16, tag="kT")
                v_t = vpool.tile([128, NKT, 130], BF16, tag="vv")
                # softmax-denominator ones columns for every k-tile in two
                # strided memsets (v_chain's psum drains skip these columns)
                nc.vector.memset(v_t[:, :, 64:65], 1.0)
                nc.vector.memset(v_t[:, :, 129:130], 1.0)
                qkv_tiles[bb] = (qT, kT, v_t)

            def qkv_chunks(bb):
                """Yield thunks, each emitting one 8-matmul qkv chain for
                batch bb.  Emitted interleaved into the previous batch's
                attention so the PE always has dense independent work."""
                qT, kT, v_t = qkv_tiles[bb]
                xt_t = xt_tiles[bb]

                def qk_chain(dst, w_t, tcn):
                    pqk = ps_m.tile([128, 512], F32, tag="ps_m")
                    for ct in range(NCT):
                        nc.tensor.matmul(
                            pqk[:],
                            w_t[:, ct, :],
                            xt_t[:, ct, tcn * 512:(tcn + 1) * 512],
                            start=(ct == 0), stop=(ct == NCT - 1),
                        )
                    # psum drains ride the scalar engine (ACT is close to
                    # PSUM and has slack; DVE carries the softmax ssts)
                    nc.scalar.copy(
                        dst[:, tcn * 512:(tcn + 1) * 512], pqk[:]
                    )

                def v_chain(kt):
                    # v (+ones cols): head slices [0:65]=[v_h0|ones] and
                    # [65:130]=[v_h1|ones] put both denominators at psum
                    # row 64
                    pv = ps_m.tile([128, 512], F32, tag="ps_m")
                    for ct in range(NCT):
                        nc.tensor.matmul(
                            pv[:, 0:130],
                            xt_t[:, ct, kt * 128:(kt + 1) * 128],
                            wv_t[:, ct, :],
                            start=(ct == 0), stop=(ct == NCT - 1),
                        )
                    nc.scalar.copy(v_t[:, kt, 0:64], pv[:, 0:64])
                    nc.scalar.copy(v_t[:, kt, 65:129], pv[:, 65:129])

                for dst, w_t in ((qT, wq_t), (kT, wk_t)):
                    for tcn in range(NQC):
                        yield lambda d=dst, w=w_t, t=tcn: qk_chain(d, w, t)
                for kt in range(NKT):
                    yield lambda k=kt: v_chain(k)

            # batch 0's qkv runs up front (nothing to overlap with)
            alloc_qkv(0)
            for chunk in qkv_chunks(0):
                chunk()
            if B > 1:
                load_xt(1, chunked=True)
                alloc_qkv(1)
            # bias tables land on the hw-dma queues behind the xt loads:
            # lb/eb tile j is first needed at attention(0) k-tile 2j (~55us
            # in), well after these ~8MB drain; wp only at the first
            # projection (end of batch 1)
            for j in range(NKT // 2):
                nc.sync.dma_start(lb_ts[j][:], lb[:, j, :])
                nc.sync.dma_start(eb_ts[j][:], eb[:, j, :])
            for ct in range(NCT):
                nc.sync.dma_start(wp_t[:, ct, :], wp_r[:, ct, :])

            # ---- per-batch attention ------------------------------------
            for b in range(B):
                qT, kT, v_t = qkv_tiles.pop(b)
                xt_tiles.pop(b)
                filler = iter(qkv_chunks(b + 1)) if b + 1 < B else iter(())

                def emit_filler(n=1):
                    for _ in range(n):
                        ch = next(filler, None)
                        if ch is not None:
                            ch()

                ln0 = dpool.tile([1, N], F32, tag="ln0")
                ln1 = dpool.tile([1, N], F32, tag="ln1")
                rc0 = dpool.tile([1, N], BF16, tag="rc0")
                rc1 = dpool.tile([1, N], BF16, tag="rc1")
                ou = opool.tile([128, N], BF16, tag="outu")
                on = onpool.tile([128, N], BF16, tag="outn")
                last = b == B - 1
                if last:
                    # the last batch's AllToAll is split into two half-token
                    # collectives (128-token chunks, all 8 chunks valid per
                    # half) so the first half's projection hides under qc2-3
                    # instead of serializing after all of attention
                    a2a_i3 = []
                    a2a_o3 = []
                    for hf in range(2):
                        a2a_i3h = drpool.tile([N_CORES, 128, 128], BF16,
                                              tag=f"a2ai{b}_{hf}")
                        a2a_o3h = drpool.tile([N_CORES, 128, 128], BF16,
                                              tag=f"a2ao{b}_{hf}")
                        a2a_i3.append(a2a_i3h)
                        a2a_o3.append(a2a_o3h)
                else:
                    a2a_i = drpool.tile([N_CORES, 128, TB], BF16,
                                        tag=f"a2ai{b}")
                    a2a_o = drpool.tile([N_CORES, 128, TB], BF16,
                                        tag=f"a2ao{b}")
                # Both heads processed together per k-tile: the two K=64
                # score matmuls occupy disjoint PE row groups (partitions
                # 0-63 / 64-127) and run concurrently.  attn@v matmuls are
                # skewed one k-tile behind the scores so the PE never waits
                # on the exp->mul chain of the current k-tile.
                for qc in range(NQC):
                    q0 = qc * 512
                    po0 = ps_o.tile([65, 512], F32, tag="ps_o")
                    po1 = ps_o.tile([65, 512], F32, tag="ps_o")
                    po = [po0, po1]
                    pend = []
                    for kt in range(NKT):
                        ps = ps_s.tile([128, 1024], F32, tag="ps_s")
                        for h in range(2):
                            nc.tensor.matmul(
                                ps[:, h * 512:(h + 1) * 512],
                                kT[h * 64:h * 64 + 64,
                                   kt * 128:(kt + 1) * 128],
                                qT[h * 64:h * 64 + 64, q0:q0 + 512],
                                start=True, stop=True,
                            )
                        if len(pend) >= 3:
                            pkt, ppw = pend.pop(0)
                            for h in range(2):
                                nc.tensor.matmul(
                                    po[h][:],
                                    v_t[:, pkt, h * 65:h * 65 + 65],
                                    ppw[:, h * 512:(h + 1) * 512],
                                    start=(pkt == 0), stop=False,
                                )
                        pw = ppool.tile([128, 1024], BF16, tag="pp")
                        if kt % 2 == 0:
                            # Schraudolph: pw bits = int16(ps*A + LB), read
                            # back as bf16 = approx exp(scale*s + bias) with
                            # exp(bias) folded into the int table; one wide
                            # DVE op, LB broadcast over the two heads
                            lbs = (lb_ts[kt // 2][:, q0:q0 + 512]
                                   .unsqueeze(1).broadcast_to([128, 2, 512]))
                            nc.vector.scalar_tensor_tensor(
                                pw[:].bitcast(I16), ps[:], A_EXP, lbs,
                                mybir.AluOpType.mult, mybir.AluOpType.add,
                            )
                        else:
                            # exact path: table exp then *exp(bias); both
                            # multiplies stay on DVE - gpsimd elementwise is
                            # ~3x slower and steals the DVE's SBUF port, and
                            # anything queued on gpsimd behind a collective
                            # head-of-line blocks attention
                            pexp = pepool.tile([128, 1024], BF16, tag="pe")
                            nc.scalar.activation(
                                pexp[:], ps[:],
                                mybir.ActivationFunctionType.Exp,
                                scale=SCALE,
                            )
                            ebs = eb_ts[kt // 2][:, q0:q0 + 512]
                            for h in range(2):
                                nc.vector.tensor_mul(
                                    pw[:, h * 512:(h + 1) * 512],
                                    pexp[:, h * 512:(h + 1) * 512],
                                    ebs,
                                )
                        pend.append((kt, pw))
                        # interleave next batch's qkv chains as PE filler
                        if (qc == 0 and kt >= 9 and kt % 2 == 1) or \
                           (qc > 0 and kt % 3 == 2):
                            emit_filler(1)
                    for pkt, ppw in pend:
                        for h in range(2):
                            nc.tensor.matmul(
                                po[h][:],
                                v_t[:, pkt, h * 65:h * 65 + 65],
                                ppw[:, h * 512:(h + 1) * 512],
                                start=False, stop=(pkt == NKT - 1),
                            )
                    for h, lnd, rcd in ((0, ln0, rc0), (1, ln1, rc1)):
                        nc.vector.tensor_copy(
                            ou[h * 64:(h + 1) * 64, q0:q0 + 512],
                            po[h][0:64, :],
                        )
                        nc.scalar.activation(
                            lnd[0:1, q0:q0 + 512], po[h][64:65, :],
                            mybir.ActivationFunctionType.Ln,
                        )
                        # 1/den = exp(-ln den), one row per head
                        nc.scalar.activation(
                            rcd[0:1, q0:q0 + 512], lnd[0:1, q0:q0 + 512],
                            mybir.ActivationFunctionType.Exp, scale=-1.0,
                        )
                    # broadcast each head's 1/den row to its 64-partition
                    # range via two accumulating K=1 indicator matmuls, then
                    # normalize reading the broadcast straight from PSUM
                    pb = ps_o.tile([128, 512], F32, tag="ps_o")
                    nc.tensor.matmul(
                        pb[:], ind0[:], rc0[0:1, q0:q0 + 512],
                        start=True, stop=False,
                    )
                    nc.tensor.matmul(
                        pb[:], ind1[:], rc1[0:1, q0:q0 + 512],
                        start=False, stop=True,
                    )
                    nc.vector.tensor_mul(
                        on[:, q0:q0 + 512], ou[:, q0:q0 + 512], pb[:]
                    )
                    if last:
                        hf, sub = qc // 2, qc % 2
                        for j in range(4):
                            nc.sync.dma_start(
                                a2a_i3[hf][sub * 4 + j, :, :],
                                on[:, q0 + j * 128:q0 + (j + 1) * 128],
                            )
                        if qc == 1:
                            nc.gpsimd.collective_compute(
                                "AllToAll",
                                mybir.AluOpType.bypass,
                                replica_groups=[list(range(N_CORES))],
                                ins=[a2a_i3[0].opt()],
                                outs=[a2a_o3[0].opt()],
                            )
                        # spread the previous batch's projection over the
                        # filler-less last batch, then slot the first half
                        # projection of this batch under qc3
                        if pending_proj is not None and qc in (1, 2):
                            emit_proj_part(
                                pending_proj[1], pending_proj[0] * TB, TB,
                                [0, 1, 2, 3] if qc == 1 else [4, 5, 6, 7],
                            )
                            if qc == 2:
                                pending_proj = None
                        if qc == 3:
                            emit_proj_part(a2a_o3[0], b * TB, 128,
                                           list(range(NCT)), gat_tag="gat3")
                    else:
                        for j in (2 * qc, 2 * qc + 1):
                            nc.sync.dma_start(
                                a2a_i[j, :, :], on[:, j * TB:(j + 1) * TB]
                            )
                        if qc == 3 and pending_proj is not None:
                            # previous batch's projection, emitted as late as
                            # possible: its gather depends on that batch's
                            # AllToAll, and scheduling it early lets the PE's
                            # static instruction order head-of-line block on
                            # the collective at the batch boundary
                            emit_proj(*pending_proj)
                            pending_proj = None
                    emit_filler(2)

                emit_filler(NKT + 2 * NQC)  # flush any remaining chunks
                if last:
                    nc.gpsimd.collective_compute(
                        "AllToAll",
                        mybir.AluOpType.bypass,
                        replica_groups=[list(range(N_CORES))],
                        ins=[a2a_i3[1].opt()],
                        outs=[a2a_o3[1].opt()],
                    )
                else:
                    nc.gpsimd.collective_compute(
                        "AllToAll",
                        mybir.AluOpType.bypass,
                        replica_groups=[list(range(N_CORES))],
                        ins=[a2a_i.opt()],
                        outs=[a2a_o.opt()],
                    )
                    pending_proj = (b, a2a_o)
                if b + 2 < B:
                    load_xt(b + 2)
                    alloc_qkv(b + 2)

            emit_proj_part(a2a_o3[1], (B - 1) * TB + 128, 128,
                           list(range(NCT)), gat_tag="gat3b")
    nc.compile()
    return nc


def _graph():
    global _GRAPH
    if _GRAPH is None:
        _GRAPH = _build()
    return _GRAPH


def _prep_inputs(x, W_qkv, W_proj, b_proj, global_bias):
    x = np.asarray(x, dtype=np.float32)
    W_qkv = np.asarray(W_qkv, dtype=np.float32)
    W_proj = np.asarray(W_proj, dtype=np.float32)
    b_proj = np.asarray(b_proj, dtype=np.float32)
    global_bias = np.asarray(global_bias, dtype=np.float32)

    xt = np.ascontiguousarray(x.reshape(TOK, C).T).astype(BF16_NP)
    wp = np.ascontiguousarray(W_proj.T).astype(BF16_NP)
    bpv = np.ascontiguousarray(b_proj[:, None])
    ebt = np.exp(global_bias).T  # [k, q]
    eb_kt = ebt.reshape(NKT, 128, N)
    ebp = np.ascontiguousarray(
        eb_kt[1::2].transpose(1, 0, 2)
    ).astype(BF16_NP)
    # Schraudolph int table for even k-tiles: bf16 bits of exp(bias) shifted
    # into the exponent field, to be added to A_EXP*scores
    lbt = np.round(
        16256.0 - C_ADJ + (128 * np.log2(np.e)) * global_bias
    ).astype(np.int16).T  # [k, q]
    lbp = np.ascontiguousarray(
        lbt.reshape(NKT, 128, N)[0::2].transpose(1, 0, 2)
    )

    in_maps = []
    for c in range(N_CORES):
        r0 = c * 128
        wq_c = np.ascontiguousarray(W_qkv[r0:r0 + 128, :].T).astype(BF16_NP)
        wk_c = np.ascontiguousarray(W_qkv[C + r0:C + r0 + 128, :].T).astype(BF16_NP)
        vt = W_qkv[2 * C + r0:2 * C + r0 + 128, :].T  # [C, 128]
        wv_c = np.zeros((C, 130), dtype=np.float32)
        wv_c[:, 0:64] = vt[:, 0:64]
        wv_c[:, 65:129] = vt[:, 64:128]
        in_maps.append({
            "xt": xt,
            "wq": wq_c,
            "wk": wk_c,
            "wv": wv_c.astype(BF16_NP),
            "wp": wp,
            "bp": bpv,
            "lb": lbp,
            "eb": ebp,
        })
    return in_maps


def _assemble(results):
    full = np.empty((TOK, C), dtype=np.float32)
    for c in range(N_CORES):
        o = results[c]["out"].T  # [4*TB tokens, C]
        for b in range(B - 1):
            full[b * N + c * TB:b * N + (c + 1) * TB, :] = (
                o[b * TB:(b + 1) * TB, :]
            )
        # last batch was shipped as two half-token AllToAlls with
        # 128-token chunks: core c holds tokens [c*128, (c+1)*128) and
        # [1024 + c*128, 1024 + (c+1)*128)
        b = B - 1
        full[b * N + c * 128:b * N + (c + 1) * 128, :] = (
            o[b * TB:b * TB + 128, :]
        )
        full[b * N + 1024 + c * 128:b * N + 1024 + (c + 1) * 128, :] = (
            o[b * TB + 128:b * TB + 256, :]
        )
    return full.reshape(B, N, C)


def kernel(x, W_qkv, W_proj, b_proj, global_bias):
    nc = _graph()
    in_maps = _prep_inputs(x, W_qkv, W_proj, b_proj, global_bias)
    res = run_bass_kernel_spmd(nc, in_maps, core_ids=list(range(N_CORES)))
    return _assemble(res.results)


def run_profiled(x, W_qkv, W_proj, b_proj, global_bias, **trace_kwargs):
    """Like kernel() but with NTFF profiling; returns (output, results)."""
    nc = _graph()
    in_maps = _prep_inputs(x, W_qkv, W_proj, b_proj, global_bias)
    res = run_bass_kernel_spmd(
        nc, in_maps, core_ids=list(range(N_CORES)), trace=True, **trace_kwargs
    )
    return _assemble(res.results), res

